# revision 12
# baseline (speedup 1.0000x reference)
"""Trainium2 Bass kernel for a DGC-VAE (GCN encoder + inner-product decoder).

reference:
    h      = relu(GCN(x, W1, b1))
    mu     = GCN(h, W_mu, b_mu);  logvar = GCN(h, W_lv, b_lv)
    z      = mu + eps * exp(0.5 * logvar)
    recon  = sigmoid(z @ z.T)
    returns (recon, mu, logvar)

GCN(x, W) with symmetric norm and self-loops rewrites as
    out[d] = dinv[d] * (sum_{e: src_e->d} Xs[src_e] + Xs[d]) + b,
    Xs = dinv (row-)scaled (x @ W),  dinv = rsqrt(indeg + 1)
so per-edge weights disappear; aggregation = row gather (dma_gather) + 0/1
selection-matrix matmuls accumulated in PSUM.

Sharding: data-parallel over nodes, 8 cores x 2048 rows. Gather sources
(Xs, Hs tables) and zT are AllGathered. Each core computes a [2048, 16384]
block of recon. On-device compute is feature-major ("T-land") so no PE
transposes are needed in the hot loops; the host passes x/eps transposed and
transposes muT/logvarT back during unsharding.
"""
import os
import sys

for _p in ("/opt/trn_rl_repo", "/root/.axon_site/_ro/trn_rl_repo"):
    if os.path.isdir(_p) and _p not in sys.path:
        sys.path.insert(0, _p)

import numpy as np

import concourse.bacc as bacc
import concourse.bass as bass  # noqa: F401
import concourse.mybir as mybir
import concourse.tile as tile
from concourse.bass_utils import run_bass_kernel_spmd
from concourse.masks import make_identity

# Problem sizes (fixed by the task)
N, IN_C, HID, LAT, E = 16384, 256, 128, 64, 262144
M = 8                 # cores
L = N // M            # rows per core
P = 128               # partitions / tile edge
T = L // P            # dst tiles per core
KI = IN_C // P        # input-channel chunks

F32 = mybir.dt.float32
BF16 = mybir.dt.bfloat16
I16 = mybir.dt.int16
I32 = mybir.dt.int32

_cache = {}
K_PAD_MIN = 0          # test hook: force larger per-tile padding
DMA_SCRATCH = 16384    # SWDGE descriptor-ring carveout (bytes)
G_SUB = 512            # max indices per dma_gather (ring-capacity limit)
RECON_BF16 = True      # compute z@z.T in bf16 (8x PE rate; recon rel err ~1e-3)
TABLE_BF16 = True      # bf16 gather tables + selection matmuls
N_QUEUES = 2           # SWDGE queues for dma_gather pipelining


def _build_program(k_pad: int):
    """Build the SPMD Bass program. k_pad = padded edges per dst tile."""
    kc = k_pad // P  # matmul chunks per tile
    kw = k_pad // 16  # wrapped idx columns per tile
    nc = bacc.Bacc("TRN2", target_bir_lowering=False, debug=False, num_devices=M,
                   dynamic_dma_scratch_size=DMA_SCRATCH,
                   num_swdge_queues=N_QUEUES)

    # ---- I/O ----
    xT_in = nc.dram_tensor("xT_in", [P, KI * L], F32, kind="ExternalInput")
    epsT_in = nc.dram_tensor("epsT_in", [LAT, L], F32, kind="ExternalInput")
    deg_in = nc.dram_tensor("deg_in", [P, L], F32, kind="ExternalInput")
    idx_in = nc.dram_tensor("idx_in", [P, T * kw], I16, kind="ExternalInput")
    dloc_in = nc.dram_tensor("dloc_in", [P, T * kc], F32, kind="ExternalInput")
    w1_in = nc.dram_tensor("w1_in", [P, KI * HID], F32, kind="ExternalInput")
    wmulv_in = nc.dram_tensor("wmulv_in", [HID, 2 * LAT], F32,
                             kind="ExternalInput")
    b1_in = nc.dram_tensor("b1_in", [HID, 1], F32, kind="ExternalInput")
    bmu_in = nc.dram_tensor("bmu_in", [LAT, 1], F32, kind="ExternalInput")
    blv_in = nc.dram_tensor("blv_in", [LAT, 1], F32, kind="ExternalInput")

    recon_out = nc.dram_tensor("recon_out", [L, N], F32, kind="ExternalOutput")
    muT_out = nc.dram_tensor("muT_out", [LAT, L], F32, kind="ExternalOutput")
    lvT_out = nc.dram_tensor("lvT_out", [LAT, L], F32, kind="ExternalOutput")

    groups = [list(range(M))]

    with tile.TileContext(nc) as tc:
        with (
            tc.tile_pool(name="const", bufs=1) as cst,
            tc.tile_pool(name="dram", bufs=1, space="DRAM") as dram,
        ):
            # ---------- constants / persistents ----------
            identity = cst.tile([P, P], F32)
            make_identity(nc, identity[:])
            iota_i = cst.tile([P, P], I32)
            nc.gpsimd.iota(iota_i[:], pattern=[[1, P]], base=0,
                           channel_multiplier=0)
            iota_f = cst.tile([P, P], F32)
            nc.vector.tensor_copy(iota_f[:], iota_i[:])

            w1 = cst.tile([P, KI, HID], F32)   # w1[p,k,:] = W1[k*128+p,:]
            nc.sync.dma_start(
                w1[:], w1_in[:].rearrange("p (k h) -> p k h", k=KI))
            wmulv = cst.tile([HID, 2 * LAT], F32)
            nc.sync.dma_start(wmulv[:], wmulv_in[:])
            b1c = cst.tile([HID, 1], F32)
            nc.sync.dma_start(b1c[:], b1_in[:])
            bmuc = cst.tile([LAT, 1], F32)
            nc.sync.dma_start(bmuc[:], bmu_in[:])
            blvc = cst.tile([LAT, 1], F32)
            nc.sync.dma_start(blvc[:], blv_in[:])
            blvh = cst.tile([LAT, 1], F32)
            nc.vector.tensor_scalar_mul(blvh[:], blvc[:], 0.5)

            dinv = cst.tile([P, L], F32)
            idxs = cst.tile([P, T * kw], I16)
            nc.sync.dma_start(idxs[:], idx_in[:])
            dloc = cst.tile([P, T * kc], F32)
            nc.sync.dma_start(dloc[:], dloc_in[:])

            xsT = cst.tile([P, L], F32)    # dinv * (x @ W1), feature-major
            hsT = cst.tile([P, L], F32)    # dinv * h, feature-major
            zT = cst.tile([LAT, L], F32)

            # DRAM: AllGather bounce buffers
            TDT = BF16 if TABLE_BF16 else F32
            xs_ag_in = dram.tile([L, HID], TDT)
            xs_full = dram.tile([N, HID], TDT, addr_space="Shared")
            hs_ag_in = dram.tile([L, HID], TDT)
            hs_full = dram.tile([N, HID], TDT, addr_space="Shared")
            ZDT = BF16 if RECON_BF16 else F32
            z_ag_in = dram.tile([LAT, L], ZDT)
            z_ag_out = dram.tile([M * LAT, L], ZDT, addr_space="Shared")

            # ---------- phase 1: Xs = dinv * (x @ W1); AllGather ----------
            with (
                tc.tile_pool(name="ph1", bufs=1) as p1,
                tc.tile_pool(name="wk1", bufs=3) as wk,
                tc.tile_pool(name="ps1", bufs=2, space="PSUM") as ps,
            ):
                deg = p1.tile([P, L], F32)
                nc.sync.dma_start(deg[:], deg_in[:])
                nc.scalar.activation(deg[:], deg[:],
                                     mybir.ActivationFunctionType.Sqrt)
                nc.vector.reciprocal(dinv[:], deg[:])
                xT = p1.tile([P, KI, L], F32)
                nc.sync.dma_start(
                    xT[:], xT_in[:].rearrange("p (k n) -> p k n", k=KI))
                for t in range(T):
                    cols = slice(t * P, (t + 1) * P)
                    xt_ps = ps.tile([P, P], F32, tag="mm")
                    for k in range(KI):
                        nc.tensor.matmul(
                            xt_ps[:], w1[:, k, :], xT[:, k, cols],
                            start=(k == 0), stop=(k == KI - 1))
                    nc.vector.tensor_tensor(out=xsT[:, cols], in0=xt_ps[:],
                                            in1=dinv[:, cols],
                                            op=mybir.AluOpType.mult)
                    # node-major copy for the gather table
                    nm_ps = ps.tile([P, P], F32, tag="tr")
                    nc.tensor.transpose(nm_ps[:], xsT[:, cols], identity[:])
                    nm = wk.tile([P, P], TDT, tag="nm")
                    nc.vector.tensor_copy(nm[:], nm_ps[:])
                    nc.sync.dma_start(xs_ag_in[t * P:(t + 1) * P, :], nm[:])
                nc.gpsimd.collective_compute(
                    "AllGather", mybir.AluOpType.bypass, replica_groups=groups,
                    ins=[xs_ag_in[:].opt()], outs=[xs_full[:].opt()])

            # ---------- phases 2+3: aggregations ----------
            with (
                tc.tile_pool(name="ph23", bufs=1) as p23,
                tc.tile_pool(name="gath", bufs=3) as gp,
                tc.tile_pool(name="wk23", bufs=3) as wk,
                tc.tile_pool(name="ps23", bufs=2, space="PSUM") as ps,
            ):
                epsT = p23.tile([LAT, L], F32)
                nc.sync.dma_start(epsT[:], epsT_in[:])
                muT = p23.tile([LAT, L], F32)
                lvT = p23.tile([LAT, L], F32)

                scache = p23.tile([P, T * kc, P], TDT)

                def sel_build(col):
                    """S[e, d] = (dloc[e] == d) for one 128-edge chunk."""
                    s = scache[:, col, :]
                    nc.vector.tensor_tensor(
                        out=s,
                        in0=dloc[:, col:col + 1].to_broadcast([P, P]),
                        in1=iota_f[:], op=mybir.AluOpType.is_equal)
                    return s

                # phase 2: h = relu(gcn1); Hs = dinv*h; AllGather
                for t in range(T):
                    cols = slice(t * P, (t + 1) * P)
                    g = gp.tile([P, kc, HID], TDT, tag="gather")
                    for si, i0 in enumerate(range(0, k_pad, G_SUB)):
                        cnt = min(G_SUB, k_pad - i0)
                        nc.gpsimd.dma_gather(
                            g[:, i0 // P:(i0 + cnt) // P, :], xs_full[:],
                            idxs[:, t * kw + i0 // 16:t * kw + (i0 + cnt) // 16],
                            cnt, cnt, HID, queue_num=(t * 8 + si) % N_QUEUES)
                    agg = ps.tile([P, P], F32, tag="mm")
                    for k in range(kc):
                        s = sel_build(t * kc + k)
                        nc.tensor.matmul(agg[:], g[:, k, :], s,
                                         start=(k == 0), stop=(k == kc - 1))
                    t1 = wk.tile([P, P], F32, tag="t1")
                    nc.vector.tensor_tensor(out=t1[:], in0=agg[:],
                                            in1=xsT[:, cols],
                                            op=mybir.AluOpType.add)
                    nc.vector.tensor_tensor(out=t1[:], in0=t1[:],
                                            in1=dinv[:, cols],
                                            op=mybir.AluOpType.mult)
                    ht = wk.tile([P, P], F32, tag="ht")
                    nc.scalar.activation(ht[:], t1[:],
                                         mybir.ActivationFunctionType.Relu,
                                         bias=b1c[:])
                    nc.vector.tensor_tensor(out=hsT[:, cols], in0=ht[:],
                                            in1=dinv[:, cols],
                                            op=mybir.AluOpType.mult)
                    nm_ps = ps.tile([P, P], F32, tag="tr")
                    nc.tensor.transpose(nm_ps[:], hsT[:, cols], identity[:])
                    nm = wk.tile([P, P], TDT, tag="nm")
                    nc.vector.tensor_copy(nm[:], nm_ps[:])
                    nc.sync.dma_start(hs_ag_in[t * P:(t + 1) * P, :], nm[:])
                nc.gpsimd.collective_compute(
                    "AllGather", mybir.AluOpType.bypass, replica_groups=groups,
                    ins=[hs_ag_in[:].opt()], outs=[hs_full[:].opt()])

                # phase 3: mu / logvar / z
                for t in range(T):
                    cols = slice(t * P, (t + 1) * P)
                    g = gp.tile([P, kc, HID], TDT, tag="gather")
                    for si, i0 in enumerate(range(0, k_pad, G_SUB)):
                        cnt = min(G_SUB, k_pad - i0)
                        nc.gpsimd.dma_gather(
                            g[:, i0 // P:(i0 + cnt) // P, :], hs_full[:],
                            idxs[:, t * kw + i0 // 16:t * kw + (i0 + cnt) // 16],
                            cnt, cnt, HID, queue_num=(t * 8 + si) % N_QUEUES)
                    agg = ps.tile([P, P], F32, tag="mm")
                    for k in range(kc):
                        s = scache[:, t * kc + k, :]
                        nc.tensor.matmul(agg[:], g[:, k, :], s,
                                         start=(k == 0), stop=(k == kc - 1))
                    gt = wk.tile([P, P], F32, tag="t1")
                    nc.vector.tensor_tensor(out=gt[:], in0=agg[:],
                                            in1=hsT[:, cols],
                                            op=mybir.AluOpType.add)
                    nc.vector.tensor_tensor(out=gt[:], in0=gt[:],
                                            in1=dinv[:, cols],
                                            op=mybir.AluOpType.mult)
                    mulv_ps = ps.tile([2 * LAT, P], F32, tag="mulv")
                    nc.tensor.matmul(mulv_ps[:], wmulv[:], gt[:], start=True,
                                     stop=True)
                    nc.vector.tensor_scalar_add(muT[:, cols],
                                                mulv_ps[:LAT, :], bmuc[:])
                    nc.vector.tensor_scalar_add(lvT[:, cols],
                                                mulv_ps[LAT:, :], blvc[:])
                    sd = wk.tile([LAT, P], F32, tag="sd")
                    nc.scalar.activation(sd[:], mulv_ps[LAT:, :],
                                         mybir.ActivationFunctionType.Exp,
                                         bias=blvh[:], scale=0.5)
                    nc.vector.tensor_tensor(out=sd[:], in0=epsT[:, cols],
                                            in1=sd[:],
                                            op=mybir.AluOpType.mult)
                    nc.vector.tensor_tensor(out=zT[:, cols],
                                            in0=muT[:, cols], in1=sd[:],
                                            op=mybir.AluOpType.add)
                nc.sync.dma_start(muT_out[:], muT[:])
                nc.sync.dma_start(lvT_out[:], lvT[:])
                if RECON_BF16:
                    zc = cst.tile([LAT, L], BF16)
                    nc.vector.tensor_copy(zc[:], zT[:])
                else:
                    zc = zT
                nc.sync.dma_start(z_ag_in[:], zc[:])
                nc.gpsimd.collective_compute(
                    "AllGather", mybir.AluOpType.bypass, replica_groups=groups,
                    ins=[z_ag_in[:].opt()], outs=[z_ag_out[:].opt()])

            # ---------- phase 4: recon = sigmoid(z @ z.T) block rows ------
            with (
                tc.tile_pool(name="ph4", bufs=1) as p4,
                tc.tile_pool(name="stage", bufs=2) as stg,
                tc.tile_pool(name="ps4", bufs=2, space="PSUM") as ps,
            ):
                zT_full = p4.tile([LAT, N], ZDT)
                nc.sync.dma_start(
                    zT_full[:].rearrange("f (r n) -> f r n", r=M),
                    z_ag_out[:].rearrange("(r f) n -> f r n", r=M))
                HALF = N // 2
                WID = 2048
                for t in range(T):
                    for half in range(2):
                        row = stg.tile([P, HALF], F32, tag="stage")
                        for jj in range(HALF // WID):
                            r_ps = ps.tile([P, WID], F32, tag="recon")
                            for j in range(WID // 512):
                                c0 = half * HALF + jj * WID + j * 512
                                nc.tensor.matmul(
                                    r_ps[:, j * 512:(j + 1) * 512],
                                    zc[:, t * P:(t + 1) * P],
                                    zT_full[:, c0:c0 + 512],
                                    start=True, stop=True)
                            nc.scalar.activation(
                                row[:, jj * WID:(jj + 1) * WID], r_ps[:],
                                mybir.ActivationFunctionType.Sigmoid)
                        nc.sync.dma_start(
                            recon_out[t * P:(t + 1) * P,
                                      half * HALF:(half + 1) * HALF], row[:])
    nc.compile()
    return nc


def _shard(x, edge_index, eps, W1, b1, W_mu, b_mu, W_lv, b_lv):
    """Host-side sharding: per-core input maps + padded gather indices."""
    src = edge_index[0].astype(np.int64)
    dst = edge_index[1].astype(np.int64)
    deg = np.bincount(dst, minlength=N).astype(np.float32) + 1.0

    order = np.argsort(dst, kind="stable")
    src_s = src[order]
    dst_s = dst[order]
    gtile = dst_s // P                       # global dst tile of each edge
    counts = np.bincount(gtile, minlength=N // P)
    k_pad = max(P, int(-(-counts.max() // P)) * P, K_PAD_MIN)

    n_tiles = N // P
    src_pad = np.zeros((n_tiles, k_pad), np.int16)
    dloc_pad = np.full((n_tiles, k_pad), 999.0, np.float32)
    starts = np.zeros(n_tiles + 1, np.int64)
    np.cumsum(counts, out=starts[1:])
    for g in range(n_tiles):
        s, e = starts[g], starts[g + 1]
        src_pad[g, : e - s] = src_s[s:e]
        dloc_pad[g, : e - s] = (dst_s[s:e] % P).astype(np.float32)

    w1h = np.ascontiguousarray(
        W1.reshape(KI, P, HID).transpose(1, 0, 2).reshape(P, KI * HID))

    in_maps = []
    for c in range(M):
        rows = slice(c * L, (c + 1) * L)
        tl = slice(c * T, (c + 1) * T)
        # idx wrap: element i of each tile at [i%16, i//16], tiles along free
        w = src_pad[tl].reshape(T, k_pad // 16, 16).transpose(2, 0, 1)
        w = np.ascontiguousarray(w.reshape(16, T * (k_pad // 16)))
        w = np.tile(w, (8, 1))
        dl = dloc_pad[tl].reshape(T, k_pad // P, P).transpose(2, 0, 1)
        dl = np.ascontiguousarray(dl.reshape(P, T * (k_pad // P)))
        xt = np.ascontiguousarray(
            x[rows].T.reshape(KI, P, L).transpose(1, 0, 2).reshape(P, KI * L))
        in_maps.append({
            "xT_in": xt,
            "epsT_in": np.ascontiguousarray(eps[rows].T),
            "deg_in": np.broadcast_to(deg[rows], (P, L)).copy(),
            "idx_in": w,
            "dloc_in": dl,
            "w1_in": w1h,
            "wmulv_in": np.ascontiguousarray(
                np.concatenate([W_mu, W_lv], axis=1)),
            "b1_in": np.ascontiguousarray(b1.reshape(HID, 1)),
            "bmu_in": np.ascontiguousarray(b_mu.reshape(LAT, 1)),
            "blv_in": np.ascontiguousarray(b_lv.reshape(LAT, 1)),
        })
    return in_maps, k_pad


def kernel(x, edge_index, eps, W1, b1, W_mu, b_mu, W_lv, b_lv, trace=False):
    in_maps, k_pad = _shard(
        np.asarray(x, np.float32), np.asarray(edge_index),
        np.asarray(eps, np.float32), np.asarray(W1, np.float32),
        np.asarray(b1, np.float32), np.asarray(W_mu, np.float32),
        np.asarray(b_mu, np.float32), np.asarray(W_lv, np.float32),
        np.asarray(b_lv, np.float32))
    if k_pad not in _cache:
        _cache[k_pad] = _build_program(k_pad)
    nc = _cache[k_pad]
    res = run_bass_kernel_spmd(nc, in_maps, core_ids=list(range(M)),
                               trace=trace)
    recon = np.concatenate([r["recon_out"] for r in res.results], axis=0)
    mu = np.concatenate([r["muT_out"].T for r in res.results], axis=0)
    lv = np.concatenate([r["lvT_out"].T for r in res.results], axis=0)
    kernel.last_exec_time_ns = res.exec_time_ns
    return recon, mu, lv


# revision 13
# speedup vs baseline: 1.1353x; 1.1353x over previous
"""Trainium2 Bass kernel for a DGC-VAE (GCN encoder + inner-product decoder).

reference:
    h      = relu(GCN(x, W1, b1))
    mu     = GCN(h, W_mu, b_mu);  logvar = GCN(h, W_lv, b_lv)
    z      = mu + eps * exp(0.5 * logvar)
    recon  = sigmoid(z @ z.T)
    returns (recon, mu, logvar)

GCN(x, W) with symmetric norm and self-loops rewrites as
    out[d] = dinv[d] * (sum_{e: src_e->d} Xs[src_e] + Xs[d]) + b,
    Xs = dinv (row-)scaled (x @ W),  dinv = rsqrt(indeg + 1)
so per-edge weights disappear; aggregation = row gather (dma_gather) + 0/1
selection-matrix matmuls accumulated in PSUM.

Sharding: data-parallel over nodes, 8 cores x 2048 rows. Gather sources
(Xs, Hs tables) and zT are AllGathered. Each core computes a [2048, 16384]
block of recon. On-device compute is feature-major ("T-land") so no PE
transposes are needed in the hot loops; the host passes x/eps transposed and
transposes muT/logvarT back during unsharding.
"""
import os
import sys

for _p in ("/opt/trn_rl_repo", "/root/.axon_site/_ro/trn_rl_repo"):
    if os.path.isdir(_p) and _p not in sys.path:
        sys.path.insert(0, _p)

import numpy as np

import concourse.bacc as bacc
import concourse.bass as bass  # noqa: F401
import concourse.mybir as mybir
import concourse.tile as tile
from concourse.bass_utils import run_bass_kernel_spmd
from concourse.masks import make_identity

# Problem sizes (fixed by the task)
N, IN_C, HID, LAT, E = 16384, 256, 128, 64, 262144
M = 8                 # cores
L = N // M            # rows per core
P = 128               # partitions / tile edge
T = L // P            # dst tiles per core
KI = IN_C // P        # input-channel chunks

F32 = mybir.dt.float32
BF16 = mybir.dt.bfloat16
I16 = mybir.dt.int16
I32 = mybir.dt.int32

_cache = {}
K_PAD_MIN = 0          # test hook: force larger per-tile padding
DMA_SCRATCH = 16384    # SWDGE descriptor-ring carveout (bytes)
G_SUB = 512            # max indices per dma_gather (ring-capacity limit)
RECON_BF16 = True      # compute z@z.T in bf16 (8x PE rate; recon rel err ~1e-3)
TABLE_BF16 = True      # bf16 gather tables + selection matmuls
N_QUEUES = 2           # SWDGE queues for dma_gather pipelining


def _build_program(k_pad: int):
    """Build the SPMD Bass program. k_pad = padded edges per dst tile."""
    kc = k_pad // P  # matmul chunks per tile
    kw = k_pad // 16  # wrapped idx columns per tile
    nc = bacc.Bacc("TRN2", target_bir_lowering=False, debug=False, num_devices=M,
                   dynamic_dma_scratch_size=DMA_SCRATCH,
                   num_swdge_queues=N_QUEUES)

    # ---- I/O ----
    xT_in = nc.dram_tensor("xT_in", [P, KI * L], F32, kind="ExternalInput")
    epsT_in = nc.dram_tensor("epsT_in", [LAT, L], F32, kind="ExternalInput")
    deg_in = nc.dram_tensor("deg_in", [P, L], F32, kind="ExternalInput")
    idx_in = nc.dram_tensor("idx_in", [P, T * kw], I16, kind="ExternalInput")
    dloc_in = nc.dram_tensor("dloc_in", [P, T * kc], F32, kind="ExternalInput")
    w1_in = nc.dram_tensor("w1_in", [P, KI * HID], F32, kind="ExternalInput")
    wmulv_in = nc.dram_tensor("wmulv_in", [HID, 2 * LAT], F32,
                             kind="ExternalInput")
    b1_in = nc.dram_tensor("b1_in", [HID, 1], F32, kind="ExternalInput")
    bmu_in = nc.dram_tensor("bmu_in", [LAT, 1], F32, kind="ExternalInput")
    blv_in = nc.dram_tensor("blv_in", [LAT, 1], F32, kind="ExternalInput")

    recon_out = nc.dram_tensor("recon_out", [L, N], F32, kind="ExternalOutput")
    muT_out = nc.dram_tensor("muT_out", [LAT, L], F32, kind="ExternalOutput")
    lvT_out = nc.dram_tensor("lvT_out", [LAT, L], F32, kind="ExternalOutput")

    groups = [list(range(M))]

    with tile.TileContext(nc) as tc:
        with (
            tc.tile_pool(name="const", bufs=1) as cst,
            tc.tile_pool(name="dram", bufs=1, space="DRAM") as dram,
        ):
            # ---------- constants / persistents ----------
            identity = cst.tile([P, P], F32)
            make_identity(nc, identity[:])
            iota_i = cst.tile([P, P], I32)
            nc.gpsimd.iota(iota_i[:], pattern=[[1, P]], base=0,
                           channel_multiplier=0)
            iota_f = cst.tile([P, P], F32)
            nc.vector.tensor_copy(iota_f[:], iota_i[:])

            w1 = cst.tile([P, KI, HID], F32)   # w1[p,k,:] = W1[k*128+p,:]
            nc.sync.dma_start(
                w1[:], w1_in[:].rearrange("p (k h) -> p k h", k=KI))
            wmulv = cst.tile([HID, 2 * LAT], F32)
            nc.sync.dma_start(wmulv[:], wmulv_in[:])
            b1c = cst.tile([HID, 1], F32)
            nc.sync.dma_start(b1c[:], b1_in[:])
            bmuc = cst.tile([LAT, 1], F32)
            nc.sync.dma_start(bmuc[:], bmu_in[:])
            blvc = cst.tile([LAT, 1], F32)
            nc.sync.dma_start(blvc[:], blv_in[:])
            blvh = cst.tile([LAT, 1], F32)
            nc.vector.tensor_scalar_mul(blvh[:], blvc[:], 0.5)

            dinv = cst.tile([P, L], F32)
            idxs = cst.tile([P, T * kw], I16)
            nc.sync.dma_start(idxs[:], idx_in[:])
            dloc = cst.tile([P, T * kc], F32)
            nc.sync.dma_start(dloc[:], dloc_in[:])

            xsT = cst.tile([P, L], F32)    # dinv * (x @ W1), feature-major
            hsT = cst.tile([P, L], F32)    # dinv * h, feature-major
            zT = cst.tile([LAT, L], F32)

            # DRAM: AllGather bounce buffers
            TDT = BF16 if TABLE_BF16 else F32
            xs_ag_in = dram.tile([L, HID], TDT)
            xs_full = dram.tile([N, HID], TDT, addr_space="Shared")
            hs_ag_in = dram.tile([L, HID], TDT)
            hs_full = dram.tile([N, HID], TDT, addr_space="Shared")
            ZDT = BF16 if RECON_BF16 else F32
            z_ag_in = dram.tile([LAT, L], ZDT)
            z_ag_out = dram.tile([M * LAT, L], ZDT, addr_space="Shared")

            # ---------- phase 1: Xs = dinv * (x @ W1); AllGather ----------
            with (
                tc.tile_pool(name="ph1", bufs=1) as p1,
                tc.tile_pool(name="wk1", bufs=3) as wk,
                tc.tile_pool(name="ps1", bufs=2, space="PSUM") as ps,
            ):
                deg = p1.tile([P, L], F32)
                nc.sync.dma_start(deg[:], deg_in[:])
                nc.scalar.activation(deg[:], deg[:],
                                     mybir.ActivationFunctionType.Sqrt)
                nc.vector.reciprocal(dinv[:], deg[:])
                xT = p1.tile([P, KI, L], F32)
                nc.sync.dma_start(
                    xT[:], xT_in[:].rearrange("p (k n) -> p k n", k=KI))
                for t in range(T):
                    cols = slice(t * P, (t + 1) * P)
                    xt_ps = ps.tile([P, P], F32, tag="mm")
                    for k in range(KI):
                        nc.tensor.matmul(
                            xt_ps[:], w1[:, k, :], xT[:, k, cols],
                            start=(k == 0), stop=(k == KI - 1))
                    nc.vector.tensor_tensor(out=xsT[:, cols], in0=xt_ps[:],
                                            in1=dinv[:, cols],
                                            op=mybir.AluOpType.mult)
                    # node-major copy for the gather table
                    nm_ps = ps.tile([P, P], F32, tag="tr")
                    nc.tensor.transpose(nm_ps[:], xsT[:, cols], identity[:])
                    nm = wk.tile([P, P], TDT, tag="nm")
                    nc.vector.tensor_copy(nm[:], nm_ps[:])
                    nc.sync.dma_start(xs_ag_in[t * P:(t + 1) * P, :], nm[:])
                nc.gpsimd.collective_compute(
                    "AllGather", mybir.AluOpType.bypass, replica_groups=groups,
                    ins=[xs_ag_in[:].opt()], outs=[xs_full[:].opt()])

            # ---------- phases 2+3: aggregations ----------
            with (
                tc.tile_pool(name="ph23", bufs=1) as p23,
                tc.tile_pool(name="gath", bufs=3) as gp,
                tc.tile_pool(name="wk23", bufs=3) as wk,
                tc.tile_pool(name="ps23", bufs=2, space="PSUM") as ps,
            ):
                epsT = p23.tile([LAT, L], F32)
                nc.sync.dma_start(epsT[:], epsT_in[:])
                muT = p23.tile([LAT, L], F32)
                lvT = p23.tile([LAT, L], F32)

                scache = p23.tile([P, T * kc, P], TDT)

                def sel_build(col):
                    """S[e, d] = (dloc[e] == d) for one 128-edge chunk."""
                    s = scache[:, col, :]
                    nc.vector.tensor_tensor(
                        out=s,
                        in0=dloc[:, col:col + 1].to_broadcast([P, P]),
                        in1=iota_f[:], op=mybir.AluOpType.is_equal)
                    return s

                # phase 2: h = relu(gcn1); Hs = dinv*h; AllGather
                for t in range(T):
                    cols = slice(t * P, (t + 1) * P)
                    g = gp.tile([P, kc, HID], TDT, tag="gather")
                    for si, i0 in enumerate(range(0, k_pad, G_SUB)):
                        cnt = min(G_SUB, k_pad - i0)
                        nc.gpsimd.dma_gather(
                            g[:, i0 // P:(i0 + cnt) // P, :], xs_full[:],
                            idxs[:, t * kw + i0 // 16:t * kw + (i0 + cnt) // 16],
                            cnt, cnt, HID, queue_num=(t * 8 + si) % N_QUEUES)
                    agg = ps.tile([P, P], F32, tag="mm")
                    for k in range(kc):
                        s = sel_build(t * kc + k)
                        nc.tensor.matmul(agg[:], g[:, k, :], s,
                                         start=(k == 0), stop=(k == kc - 1))
                    t1 = wk.tile([P, P], F32, tag="t1")
                    nc.vector.tensor_tensor(out=t1[:], in0=agg[:],
                                            in1=xsT[:, cols],
                                            op=mybir.AluOpType.add)
                    nc.vector.tensor_tensor(out=t1[:], in0=t1[:],
                                            in1=dinv[:, cols],
                                            op=mybir.AluOpType.mult)
                    ht = wk.tile([P, P], F32, tag="ht")
                    nc.scalar.activation(ht[:], t1[:],
                                         mybir.ActivationFunctionType.Relu,
                                         bias=b1c[:])
                    nc.vector.tensor_tensor(out=hsT[:, cols], in0=ht[:],
                                            in1=dinv[:, cols],
                                            op=mybir.AluOpType.mult)
                    nm_ps = ps.tile([P, P], F32, tag="tr")
                    nc.tensor.transpose(nm_ps[:], hsT[:, cols], identity[:])
                    nm = wk.tile([P, P], TDT, tag="nm")
                    nc.vector.tensor_copy(nm[:], nm_ps[:])
                    nc.sync.dma_start(hs_ag_in[t * P:(t + 1) * P, :], nm[:])
                nc.gpsimd.collective_compute(
                    "AllGather", mybir.AluOpType.bypass, replica_groups=groups,
                    ins=[hs_ag_in[:].opt()], outs=[hs_full[:].opt()])

                # phase 3: mu / logvar / z
                for t in range(T):
                    cols = slice(t * P, (t + 1) * P)
                    g = gp.tile([P, kc, HID], TDT, tag="gather")
                    for si, i0 in enumerate(range(0, k_pad, G_SUB)):
                        cnt = min(G_SUB, k_pad - i0)
                        nc.gpsimd.dma_gather(
                            g[:, i0 // P:(i0 + cnt) // P, :], hs_full[:],
                            idxs[:, t * kw + i0 // 16:t * kw + (i0 + cnt) // 16],
                            cnt, cnt, HID, queue_num=(t * 8 + si) % N_QUEUES)
                    agg = ps.tile([P, P], F32, tag="mm")
                    for k in range(kc):
                        s = scache[:, t * kc + k, :]
                        nc.tensor.matmul(agg[:], g[:, k, :], s,
                                         start=(k == 0), stop=(k == kc - 1))
                    gt = wk.tile([P, P], F32, tag="t1")
                    nc.vector.tensor_tensor(out=gt[:], in0=agg[:],
                                            in1=hsT[:, cols],
                                            op=mybir.AluOpType.add)
                    nc.vector.tensor_tensor(out=gt[:], in0=gt[:],
                                            in1=dinv[:, cols],
                                            op=mybir.AluOpType.mult)
                    mulv_ps = ps.tile([2 * LAT, P], F32, tag="mulv")
                    nc.tensor.matmul(mulv_ps[:], wmulv[:], gt[:], start=True,
                                     stop=True)
                    nc.vector.tensor_scalar_add(muT[:, cols],
                                                mulv_ps[:LAT, :], bmuc[:])
                    nc.vector.tensor_scalar_add(lvT[:, cols],
                                                mulv_ps[LAT:, :], blvc[:])
                    sd = wk.tile([LAT, P], F32, tag="sd")
                    nc.scalar.activation(sd[:], mulv_ps[LAT:, :],
                                         mybir.ActivationFunctionType.Exp,
                                         bias=blvh[:], scale=0.5)
                    nc.vector.tensor_tensor(out=sd[:], in0=epsT[:, cols],
                                            in1=sd[:],
                                            op=mybir.AluOpType.mult)
                    nc.vector.tensor_tensor(out=zT[:, cols],
                                            in0=muT[:, cols], in1=sd[:],
                                            op=mybir.AluOpType.add)
                nc.sync.dma_start(muT_out[:], muT[:])
                nc.sync.dma_start(lvT_out[:], lvT[:])
                if RECON_BF16:
                    zc = cst.tile([LAT, L], BF16)
                    nc.vector.tensor_copy(zc[:], zT[:])
                else:
                    zc = zT
                nc.sync.dma_start(z_ag_in[:], zc[:])
                nc.gpsimd.collective_compute(
                    "AllGather", mybir.AluOpType.bypass, replica_groups=groups,
                    ins=[z_ag_in[:].opt()], outs=[z_ag_out[:].opt()])

            # ---------- phase 4: recon = sigmoid(z @ z.T) block rows ------
            with (
                tc.tile_pool(name="ph4", bufs=1) as p4,
                tc.tile_pool(name="stage", bufs=2) as stg,
                tc.tile_pool(name="ps4", bufs=2, space="PSUM") as ps,
            ):
                zT_full = p4.tile([LAT, N], ZDT)
                nc.sync.dma_start(
                    zT_full[:].rearrange("f (r n) -> f r n", r=M),
                    z_ag_out[:].rearrange("(r f) n -> f r n", r=M))
                HALF = N // 2
                WID = min(2048, HALF)
                for t in range(T):
                    for half in range(2):
                        row = stg.tile([P, HALF], F32, tag="stage")
                        for jj in range(HALF // WID):
                            r_ps = ps.tile([P, WID], F32, tag="recon")
                            for j in range(WID // 512):
                                c0 = half * HALF + jj * WID + j * 512
                                nc.tensor.matmul(
                                    r_ps[:, j * 512:(j + 1) * 512],
                                    zc[:, t * P:(t + 1) * P],
                                    zT_full[:, c0:c0 + 512],
                                    start=True, stop=True)
                            nc.scalar.activation(
                                row[:, jj * WID:(jj + 1) * WID], r_ps[:],
                                mybir.ActivationFunctionType.Sigmoid)
                        nc.sync.dma_start(
                            recon_out[t * P:(t + 1) * P,
                                      half * HALF:(half + 1) * HALF], row[:])
    nc.compile()
    return nc


def _shard(x, edge_index, eps, W1, b1, W_mu, b_mu, W_lv, b_lv):
    """Host-side sharding: per-core input maps + padded gather indices."""
    src = edge_index[0].astype(np.int64)
    dst = edge_index[1].astype(np.int64)
    deg = np.bincount(dst, minlength=N).astype(np.float32) + 1.0

    order = np.argsort(dst, kind="stable")
    src_s = src[order]
    dst_s = dst[order]
    gtile = dst_s // P                       # global dst tile of each edge
    counts = np.bincount(gtile, minlength=N // P)
    k_pad = max(P, int(-(-counts.max() // P)) * P, K_PAD_MIN)

    n_tiles = N // P
    src_pad = np.zeros((n_tiles, k_pad), np.int16)
    dloc_pad = np.full((n_tiles, k_pad), 999.0, np.float32)
    starts = np.zeros(n_tiles + 1, np.int64)
    np.cumsum(counts, out=starts[1:])
    for g in range(n_tiles):
        s, e = starts[g], starts[g + 1]
        src_pad[g, : e - s] = src_s[s:e]
        dloc_pad[g, : e - s] = (dst_s[s:e] % P).astype(np.float32)

    w1h = np.ascontiguousarray(
        W1.reshape(KI, P, HID).transpose(1, 0, 2).reshape(P, KI * HID))

    in_maps = []
    for c in range(M):
        rows = slice(c * L, (c + 1) * L)
        tl = slice(c * T, (c + 1) * T)
        # idx wrap: element i of each tile at [i%16, i//16], tiles along free
        w = src_pad[tl].reshape(T, k_pad // 16, 16).transpose(2, 0, 1)
        w = np.ascontiguousarray(w.reshape(16, T * (k_pad // 16)))
        w = np.tile(w, (8, 1))
        dl = dloc_pad[tl].reshape(T, k_pad // P, P).transpose(2, 0, 1)
        dl = np.ascontiguousarray(dl.reshape(P, T * (k_pad // P)))
        xt = np.ascontiguousarray(
            x[rows].T.reshape(KI, P, L).transpose(1, 0, 2).reshape(P, KI * L))
        in_maps.append({
            "xT_in": xt,
            "epsT_in": np.ascontiguousarray(eps[rows].T),
            "deg_in": np.broadcast_to(deg[rows], (P, L)).copy(),
            "idx_in": w,
            "dloc_in": dl,
            "w1_in": w1h,
            "wmulv_in": np.ascontiguousarray(
                np.concatenate([W_mu, W_lv], axis=1)),
            "b1_in": np.ascontiguousarray(b1.reshape(HID, 1)),
            "bmu_in": np.ascontiguousarray(b_mu.reshape(LAT, 1)),
            "blv_in": np.ascontiguousarray(b_lv.reshape(LAT, 1)),
        })
    return in_maps, k_pad


def kernel(x, edge_index, eps, W1, b1, W_mu, b_mu, W_lv, b_lv, trace=False):
    in_maps, k_pad = _shard(
        np.asarray(x, np.float32), np.asarray(edge_index),
        np.asarray(eps, np.float32), np.asarray(W1, np.float32),
        np.asarray(b1, np.float32), np.asarray(W_mu, np.float32),
        np.asarray(b_mu, np.float32), np.asarray(W_lv, np.float32),
        np.asarray(b_lv, np.float32))
    if k_pad not in _cache:
        _cache[k_pad] = _build_program(k_pad)
    nc = _cache[k_pad]
    res = run_bass_kernel_spmd(nc, in_maps, core_ids=list(range(M)),
                               trace=trace)
    recon = np.concatenate([r["recon_out"] for r in res.results], axis=0)
    mu = np.concatenate([r["muT_out"].T for r in res.results], axis=0)
    lv = np.concatenate([r["lvT_out"].T for r in res.results], axis=0)
    kernel.last_exec_time_ns = res.exec_time_ns
    return recon, mu, lv


# revision 15
# speedup vs baseline: 1.3655x; 1.2027x over previous
"""Trainium2 Bass kernel for a DGC-VAE (GCN encoder + inner-product decoder).

reference:
    h      = relu(GCN(x, W1, b1))
    mu     = GCN(h, W_mu, b_mu);  logvar = GCN(h, W_lv, b_lv)
    z      = mu + eps * exp(0.5 * logvar)
    recon  = sigmoid(z @ z.T)
    returns (recon, mu, logvar)

GCN(x, W) with symmetric norm and self-loops rewrites as
    out[d] = dinv[d] * (sum_{e: src_e->d} Xs[src_e] + Xs[d]) + b,
    Xs = dinv (row-)scaled (x @ W),  dinv = rsqrt(indeg + 1)
so per-edge weights disappear; aggregation = row gather (dma_gather) + 0/1
selection-matrix matmuls accumulated in PSUM.

Sharding: data-parallel over nodes, 8 cores x 2048 rows. Gather sources
(Xs, Hs tables) and zT are AllGathered. Each core computes a [2048, 16384]
block of recon. On-device compute is feature-major ("T-land") so no PE
transposes are needed in the hot loops; the host passes x/eps transposed and
transposes muT/logvarT back during unsharding.
"""
import os
import sys

for _p in ("/opt/trn_rl_repo", "/root/.axon_site/_ro/trn_rl_repo"):
    if os.path.isdir(_p) and _p not in sys.path:
        sys.path.insert(0, _p)

import numpy as np

import concourse.bacc as bacc
import concourse.bass as bass  # noqa: F401
import concourse.mybir as mybir
import concourse.tile as tile
from concourse.bass_utils import run_bass_kernel_spmd
from concourse.masks import make_identity

# Problem sizes (fixed by the task)
N, IN_C, HID, LAT, E = 16384, 256, 128, 64, 262144
M = 8                 # cores
L = N // M            # rows per core
P = 128               # partitions / tile edge
T = L // P            # dst tiles per core
KI = IN_C // P        # input-channel chunks

F32 = mybir.dt.float32
BF16 = mybir.dt.bfloat16
I16 = mybir.dt.int16
I32 = mybir.dt.int32

_cache = {}
K_PAD_MIN = 0          # test hook: force larger per-tile padding
DMA_SCRATCH = 16384    # SWDGE descriptor-ring carveout (bytes)
G_SUB = 512            # max indices per dma_gather (ring-capacity limit)
RECON_BF16 = True      # compute z@z.T in bf16 (8x PE rate; recon rel err ~1e-3)
TABLE_BF16 = True      # bf16 gather tables + selection matmuls
OUT_BF16 = True        # device writes recon in bf16; host casts to f32
N_QUEUES = 4           # SWDGE queues for dma_gather pipelining


def _build_program(k_pad: int):
    """Build the SPMD Bass program. k_pad = padded edges per dst tile."""
    kc = k_pad // P  # matmul chunks per tile
    kw = k_pad // 16  # wrapped idx columns per tile
    nc = bacc.Bacc("TRN2", target_bir_lowering=False, debug=False, num_devices=M,
                   dynamic_dma_scratch_size=DMA_SCRATCH,
                   num_swdge_queues=N_QUEUES)

    # ---- I/O ----
    xT_in = nc.dram_tensor("xT_in", [P, KI * L], F32, kind="ExternalInput")
    epsT_in = nc.dram_tensor("epsT_in", [LAT, L], F32, kind="ExternalInput")
    deg_in = nc.dram_tensor("deg_in", [P, L], F32, kind="ExternalInput")
    idx_in = nc.dram_tensor("idx_in", [P, T * kw], I16, kind="ExternalInput")
    dloc_in = nc.dram_tensor("dloc_in", [P, T * kc], F32, kind="ExternalInput")
    w1_in = nc.dram_tensor("w1_in", [P, KI * HID], F32, kind="ExternalInput")
    wmulv_in = nc.dram_tensor("wmulv_in", [HID, 2 * LAT], F32,
                             kind="ExternalInput")
    b1_in = nc.dram_tensor("b1_in", [HID, 1], F32, kind="ExternalInput")
    bmu_in = nc.dram_tensor("bmu_in", [LAT, 1], F32, kind="ExternalInput")
    blv_in = nc.dram_tensor("blv_in", [LAT, 1], F32, kind="ExternalInput")

    ODT = BF16 if OUT_BF16 else F32
    recon_out = nc.dram_tensor("recon_out", [L, N], ODT, kind="ExternalOutput")
    muT_out = nc.dram_tensor("muT_out", [LAT, L], F32, kind="ExternalOutput")
    lvT_out = nc.dram_tensor("lvT_out", [LAT, L], F32, kind="ExternalOutput")

    groups = [list(range(M))]

    with tile.TileContext(nc) as tc:
        with (
            tc.tile_pool(name="const", bufs=1) as cst,
            tc.tile_pool(name="dram", bufs=1, space="DRAM") as dram,
        ):
            # ---------- constants / persistents ----------
            identity = cst.tile([P, P], F32)
            make_identity(nc, identity[:])
            iota_i = cst.tile([P, P], I32)
            nc.gpsimd.iota(iota_i[:], pattern=[[1, P]], base=0,
                           channel_multiplier=0)
            iota_f = cst.tile([P, P], F32)
            nc.vector.tensor_copy(iota_f[:], iota_i[:])

            w1 = cst.tile([P, KI, HID], F32)   # w1[p,k,:] = W1[k*128+p,:]
            nc.sync.dma_start(
                w1[:], w1_in[:].rearrange("p (k h) -> p k h", k=KI))
            wmulv = cst.tile([HID, 2 * LAT], F32)
            nc.sync.dma_start(wmulv[:], wmulv_in[:])
            b1c = cst.tile([HID, 1], F32)
            nc.sync.dma_start(b1c[:], b1_in[:])
            bmuc = cst.tile([LAT, 1], F32)
            nc.sync.dma_start(bmuc[:], bmu_in[:])
            blvc = cst.tile([LAT, 1], F32)
            nc.sync.dma_start(blvc[:], blv_in[:])
            blvh = cst.tile([LAT, 1], F32)
            nc.vector.tensor_scalar_mul(blvh[:], blvc[:], 0.5)

            dinv = cst.tile([P, L], F32)
            idxs = cst.tile([P, T * kw], I16)
            nc.sync.dma_start(idxs[:], idx_in[:])
            dloc = cst.tile([P, T * kc], F32)
            nc.sync.dma_start(dloc[:], dloc_in[:])

            xsT = cst.tile([P, L], F32)    # dinv * (x @ W1), feature-major
            hsT = cst.tile([P, L], F32)    # dinv * h, feature-major
            zT = cst.tile([LAT, L], F32)

            # DRAM: AllGather bounce buffers
            TDT = BF16 if TABLE_BF16 else F32
            xs_ag_in = dram.tile([L, HID], TDT)
            xs_full = dram.tile([N, HID], TDT, addr_space="Shared")
            hs_ag_in = dram.tile([L, HID], TDT)
            hs_full = dram.tile([N, HID], TDT, addr_space="Shared")
            ZDT = BF16 if RECON_BF16 else F32
            z_ag_in = dram.tile([LAT, L], ZDT)
            z_ag_out = dram.tile([M * LAT, L], ZDT, addr_space="Shared")

            # ---------- phase 1: Xs = dinv * (x @ W1); AllGather ----------
            with (
                tc.tile_pool(name="ph1", bufs=1) as p1,
                tc.tile_pool(name="wk1", bufs=3) as wk,
                tc.tile_pool(name="ps1", bufs=2, space="PSUM") as ps,
            ):
                deg = p1.tile([P, L], F32)
                nc.sync.dma_start(deg[:], deg_in[:])
                nc.scalar.activation(deg[:], deg[:],
                                     mybir.ActivationFunctionType.Sqrt)
                nc.vector.reciprocal(dinv[:], deg[:])
                xT = p1.tile([P, KI, L], F32)
                nc.sync.dma_start(
                    xT[:], xT_in[:].rearrange("p (k n) -> p k n", k=KI))
                for t in range(T):
                    cols = slice(t * P, (t + 1) * P)
                    xt_ps = ps.tile([P, P], F32, tag="mm")
                    for k in range(KI):
                        nc.tensor.matmul(
                            xt_ps[:], w1[:, k, :], xT[:, k, cols],
                            start=(k == 0), stop=(k == KI - 1))
                    nc.vector.tensor_tensor(out=xsT[:, cols], in0=xt_ps[:],
                                            in1=dinv[:, cols],
                                            op=mybir.AluOpType.mult)
                    # node-major copy for the gather table
                    nm_ps = ps.tile([P, P], F32, tag="tr")
                    nc.tensor.transpose(nm_ps[:], xsT[:, cols], identity[:])
                    nm = wk.tile([P, P], TDT, tag="nm")
                    nc.vector.tensor_copy(nm[:], nm_ps[:])
                    nc.sync.dma_start(xs_ag_in[t * P:(t + 1) * P, :], nm[:])
                nc.gpsimd.collective_compute(
                    "AllGather", mybir.AluOpType.bypass, replica_groups=groups,
                    ins=[xs_ag_in[:].opt()], outs=[xs_full[:].opt()])

            # ---------- phases 2+3: aggregations ----------
            with (
                tc.tile_pool(name="ph23", bufs=1) as p23,
                tc.tile_pool(name="gath", bufs=3) as gp,
                tc.tile_pool(name="wk23", bufs=3) as wk,
                tc.tile_pool(name="ps23", bufs=2, space="PSUM") as ps,
            ):
                epsT = p23.tile([LAT, L], F32)
                nc.sync.dma_start(epsT[:], epsT_in[:])
                muT = p23.tile([LAT, L], F32)
                lvT = p23.tile([LAT, L], F32)

                scache = p23.tile([P, T * kc, P], TDT)

                def sel_build(col):
                    """S[e, d] = (dloc[e] == d) for one 128-edge chunk."""
                    s = scache[:, col, :]
                    nc.vector.tensor_tensor(
                        out=s,
                        in0=dloc[:, col:col + 1].to_broadcast([P, P]),
                        in1=iota_f[:], op=mybir.AluOpType.is_equal)
                    return s

                # phase 2: h = relu(gcn1); Hs = dinv*h; AllGather
                for t in range(T):
                    cols = slice(t * P, (t + 1) * P)
                    g = gp.tile([P, kc, HID], TDT, tag="gather")
                    for si, i0 in enumerate(range(0, k_pad, G_SUB)):
                        cnt = min(G_SUB, k_pad - i0)
                        nc.gpsimd.dma_gather(
                            g[:, i0 // P:(i0 + cnt) // P, :], xs_full[:],
                            idxs[:, t * kw + i0 // 16:t * kw + (i0 + cnt) // 16],
                            cnt, cnt, HID, queue_num=(t * 8 + si) % N_QUEUES)
                    agg = ps.tile([P, P], F32, tag="mm")
                    for k in range(kc):
                        s = sel_build(t * kc + k)
                        nc.tensor.matmul(agg[:], g[:, k, :], s,
                                         start=(k == 0), stop=(k == kc - 1))
                    t1 = wk.tile([P, P], F32, tag="t1")
                    nc.vector.tensor_tensor(out=t1[:], in0=agg[:],
                                            in1=xsT[:, cols],
                                            op=mybir.AluOpType.add)
                    nc.vector.tensor_tensor(out=t1[:], in0=t1[:],
                                            in1=dinv[:, cols],
                                            op=mybir.AluOpType.mult)
                    ht = wk.tile([P, P], F32, tag="ht")
                    nc.scalar.activation(ht[:], t1[:],
                                         mybir.ActivationFunctionType.Relu,
                                         bias=b1c[:])
                    nc.vector.tensor_tensor(out=hsT[:, cols], in0=ht[:],
                                            in1=dinv[:, cols],
                                            op=mybir.AluOpType.mult)
                    nm_ps = ps.tile([P, P], F32, tag="tr")
                    nc.tensor.transpose(nm_ps[:], hsT[:, cols], identity[:])
                    nm = wk.tile([P, P], TDT, tag="nm")
                    nc.vector.tensor_copy(nm[:], nm_ps[:])
                    nc.sync.dma_start(hs_ag_in[t * P:(t + 1) * P, :], nm[:])
                nc.gpsimd.collective_compute(
                    "AllGather", mybir.AluOpType.bypass, replica_groups=groups,
                    ins=[hs_ag_in[:].opt()], outs=[hs_full[:].opt()])

                # phase 3: mu / logvar / z
                for t in range(T):
                    cols = slice(t * P, (t + 1) * P)
                    g = gp.tile([P, kc, HID], TDT, tag="gather")
                    for si, i0 in enumerate(range(0, k_pad, G_SUB)):
                        cnt = min(G_SUB, k_pad - i0)
                        nc.gpsimd.dma_gather(
                            g[:, i0 // P:(i0 + cnt) // P, :], hs_full[:],
                            idxs[:, t * kw + i0 // 16:t * kw + (i0 + cnt) // 16],
                            cnt, cnt, HID, queue_num=(t * 8 + si) % N_QUEUES)
                    agg = ps.tile([P, P], F32, tag="mm")
                    for k in range(kc):
                        s = scache[:, t * kc + k, :]
                        nc.tensor.matmul(agg[:], g[:, k, :], s,
                                         start=(k == 0), stop=(k == kc - 1))
                    gt = wk.tile([P, P], F32, tag="t1")
                    nc.vector.tensor_tensor(out=gt[:], in0=agg[:],
                                            in1=hsT[:, cols],
                                            op=mybir.AluOpType.add)
                    nc.vector.tensor_tensor(out=gt[:], in0=gt[:],
                                            in1=dinv[:, cols],
                                            op=mybir.AluOpType.mult)
                    mulv_ps = ps.tile([2 * LAT, P], F32, tag="mulv")
                    nc.tensor.matmul(mulv_ps[:], wmulv[:], gt[:], start=True,
                                     stop=True)
                    nc.vector.tensor_scalar_add(muT[:, cols],
                                                mulv_ps[:LAT, :], bmuc[:])
                    nc.vector.tensor_scalar_add(lvT[:, cols],
                                                mulv_ps[LAT:, :], blvc[:])
                    sd = wk.tile([LAT, P], F32, tag="sd")
                    nc.scalar.activation(sd[:], mulv_ps[LAT:, :],
                                         mybir.ActivationFunctionType.Exp,
                                         bias=blvh[:], scale=0.5)
                    nc.vector.tensor_tensor(out=sd[:], in0=epsT[:, cols],
                                            in1=sd[:],
                                            op=mybir.AluOpType.mult)
                    nc.vector.tensor_tensor(out=zT[:, cols],
                                            in0=muT[:, cols], in1=sd[:],
                                            op=mybir.AluOpType.add)
                nc.sync.dma_start(muT_out[:], muT[:])
                nc.sync.dma_start(lvT_out[:], lvT[:])
                if RECON_BF16:
                    zc = cst.tile([LAT, L], BF16)
                    nc.vector.tensor_copy(zc[:], zT[:])
                else:
                    zc = zT
                nc.sync.dma_start(z_ag_in[:], zc[:])
                nc.gpsimd.collective_compute(
                    "AllGather", mybir.AluOpType.bypass, replica_groups=groups,
                    ins=[z_ag_in[:].opt()], outs=[z_ag_out[:].opt()])

            # ---------- phase 4: recon = sigmoid(z @ z.T) block rows ------
            with (
                tc.tile_pool(name="ph4", bufs=1) as p4,
                tc.tile_pool(name="stage", bufs=2) as stg,
                tc.tile_pool(name="ps4", bufs=2, space="PSUM") as ps,
            ):
                zT_full = p4.tile([LAT, N], ZDT)
                nc.sync.dma_start(
                    zT_full[:].rearrange("f (r n) -> f r n", r=M),
                    z_ag_out[:].rearrange("(r f) n -> f r n", r=M))
                HALF = N // 2
                WID = min(2048, HALF)
                for t in range(T):
                    for half in range(2):
                        row = stg.tile([P, HALF], ODT, tag="stage")
                        for jj in range(HALF // WID):
                            r_ps = ps.tile([P, WID], F32, tag="recon")
                            for j in range(WID // 512):
                                c0 = half * HALF + jj * WID + j * 512
                                nc.tensor.matmul(
                                    r_ps[:, j * 512:(j + 1) * 512],
                                    zc[:, t * P:(t + 1) * P],
                                    zT_full[:, c0:c0 + 512],
                                    start=True, stop=True)
                            nc.scalar.activation(
                                row[:, jj * WID:(jj + 1) * WID], r_ps[:],
                                mybir.ActivationFunctionType.Sigmoid)
                        nc.sync.dma_start(
                            recon_out[t * P:(t + 1) * P,
                                      half * HALF:(half + 1) * HALF], row[:])
    nc.compile()
    return nc


def _shard(x, edge_index, eps, W1, b1, W_mu, b_mu, W_lv, b_lv):
    """Host-side sharding: per-core input maps + padded gather indices."""
    src = edge_index[0].astype(np.int64)
    dst = edge_index[1].astype(np.int64)
    deg = np.bincount(dst, minlength=N).astype(np.float32) + 1.0

    order = np.argsort(dst, kind="stable")
    src_s = src[order]
    dst_s = dst[order]
    gtile = dst_s // P                       # global dst tile of each edge
    counts = np.bincount(gtile, minlength=N // P)
    k_pad = max(P, int(-(-counts.max() // P)) * P, K_PAD_MIN)

    n_tiles = N // P
    src_pad = np.zeros((n_tiles, k_pad), np.int16)
    dloc_pad = np.full((n_tiles, k_pad), 999.0, np.float32)
    starts = np.zeros(n_tiles + 1, np.int64)
    np.cumsum(counts, out=starts[1:])
    for g in range(n_tiles):
        s, e = starts[g], starts[g + 1]
        src_pad[g, : e - s] = src_s[s:e]
        dloc_pad[g, : e - s] = (dst_s[s:e] % P).astype(np.float32)

    w1h = np.ascontiguousarray(
        W1.reshape(KI, P, HID).transpose(1, 0, 2).reshape(P, KI * HID))

    in_maps = []
    for c in range(M):
        rows = slice(c * L, (c + 1) * L)
        tl = slice(c * T, (c + 1) * T)
        # idx wrap: element i of each tile at [i%16, i//16], tiles along free
        w = src_pad[tl].reshape(T, k_pad // 16, 16).transpose(2, 0, 1)
        w = np.ascontiguousarray(w.reshape(16, T * (k_pad // 16)))
        w = np.tile(w, (8, 1))
        dl = dloc_pad[tl].reshape(T, k_pad // P, P).transpose(2, 0, 1)
        dl = np.ascontiguousarray(dl.reshape(P, T * (k_pad // P)))
        xt = np.ascontiguousarray(
            x[rows].T.reshape(KI, P, L).transpose(1, 0, 2).reshape(P, KI * L))
        in_maps.append({
            "xT_in": xt,
            "epsT_in": np.ascontiguousarray(eps[rows].T),
            "deg_in": np.broadcast_to(deg[rows], (P, L)).copy(),
            "idx_in": w,
            "dloc_in": dl,
            "w1_in": w1h,
            "wmulv_in": np.ascontiguousarray(
                np.concatenate([W_mu, W_lv], axis=1)),
            "b1_in": np.ascontiguousarray(b1.reshape(HID, 1)),
            "bmu_in": np.ascontiguousarray(b_mu.reshape(LAT, 1)),
            "blv_in": np.ascontiguousarray(b_lv.reshape(LAT, 1)),
        })
    return in_maps, k_pad


def kernel(x, edge_index, eps, W1, b1, W_mu, b_mu, W_lv, b_lv, trace=False):
    in_maps, k_pad = _shard(
        np.asarray(x, np.float32), np.asarray(edge_index),
        np.asarray(eps, np.float32), np.asarray(W1, np.float32),
        np.asarray(b1, np.float32), np.asarray(W_mu, np.float32),
        np.asarray(b_mu, np.float32), np.asarray(W_lv, np.float32),
        np.asarray(b_lv, np.float32))
    if k_pad not in _cache:
        _cache[k_pad] = _build_program(k_pad)
    nc = _cache[k_pad]
    res = run_bass_kernel_spmd(nc, in_maps, core_ids=list(range(M)),
                               trace=trace)
    recon = np.concatenate([r["recon_out"] for r in res.results],
                           axis=0).astype(np.float32)
    mu = np.concatenate([r["muT_out"].T for r in res.results], axis=0)
    lv = np.concatenate([r["lvT_out"].T for r in res.results], axis=0)
    kernel.last_exec_time_ns = res.exec_time_ns
    return recon, mu, lv


# revision 18
# speedup vs baseline: 1.4546x; 1.0652x over previous
"""Trainium2 Bass kernel for a DGC-VAE (GCN encoder + inner-product decoder).

reference:
    h      = relu(GCN(x, W1, b1))
    mu     = GCN(h, W_mu, b_mu);  logvar = GCN(h, W_lv, b_lv)
    z      = mu + eps * exp(0.5 * logvar)
    recon  = sigmoid(z @ z.T)
    returns (recon, mu, logvar)

GCN(x, W) with symmetric norm and self-loops rewrites as
    out[d] = dinv[d] * (sum_{e: src_e->d} Xs[src_e] + Xs[d]) + b,
    Xs = dinv (row-)scaled (x @ W),  dinv = rsqrt(indeg + 1)
so per-edge weights disappear; aggregation = row gather (dma_gather) + 0/1
selection-matrix matmuls accumulated in PSUM.

Sharding: data-parallel over nodes, 8 cores x 2048 rows. Gather sources
(Xs, Hs tables) and zT are AllGathered. Each core computes a [2048, 16384]
block of recon. On-device compute is feature-major ("T-land") so no PE
transposes are needed in the hot loops; the host passes x/eps transposed and
transposes muT/logvarT back during unsharding.
"""
import os
import sys

for _p in ("/opt/trn_rl_repo", "/root/.axon_site/_ro/trn_rl_repo"):
    if os.path.isdir(_p) and _p not in sys.path:
        sys.path.insert(0, _p)

import numpy as np

import concourse.bacc as bacc
import concourse.bass as bass  # noqa: F401
import concourse.mybir as mybir
import concourse.tile as tile
from concourse.bass_utils import run_bass_kernel_spmd
from concourse.masks import make_identity

# Problem sizes (fixed by the task)
N, IN_C, HID, LAT, E = 16384, 256, 128, 64, 262144
M = 8                 # cores
L = N // M            # rows per core
P = 128               # partitions / tile edge
T = L // P            # dst tiles per core
KI = IN_C // P        # input-channel chunks

F32 = mybir.dt.float32
BF16 = mybir.dt.bfloat16
I16 = mybir.dt.int16
I32 = mybir.dt.int32

_cache = {}
K_PAD_MIN = 0          # test hook: force larger per-tile padding
DMA_SCRATCH = 16384    # SWDGE descriptor-ring carveout (bytes)
G_SUB = 512            # max indices per dma_gather (ring-capacity limit)
RECON_BF16 = True      # compute z@z.T in bf16 (8x PE rate; recon rel err ~1e-3)
TABLE_BF16 = True      # bf16 gather tables + selection matmuls
OUT_BF16 = True        # device writes recon in bf16; host casts to f32
N_QUEUES = 4           # SWDGE queues for dma_gather pipelining


def _build_program(ks: tuple):
    """Build the SPMD Bass program. ks[t] = padded edge count of dst tile t
    (multiple of 16, max over cores, shared by all cores for SPMD)."""
    kcs = [-(-k // P) for k in ks]          # matmul chunks per tile
    kws = [k // 16 for k in ks]             # wrapped idx cols per tile
    iofs = [0]
    for k in ks:
        iofs.append(iofs[-1] + k // 16)     # idx col offset per tile
    kc = max(kcs)                           # gather-tile chunk capacity
    kw_tot = iofs[-1]
    nc = bacc.Bacc("TRN2", target_bir_lowering=False, debug=False, num_devices=M,
                   dynamic_dma_scratch_size=DMA_SCRATCH,
                   num_swdge_queues=N_QUEUES)

    # ---- I/O ----
    xT_in = nc.dram_tensor("xT_in", [P, KI * L], F32, kind="ExternalInput")
    epsT_in = nc.dram_tensor("epsT_in", [LAT, L], F32, kind="ExternalInput")
    deg_in = nc.dram_tensor("deg_in", [P, L], F32, kind="ExternalInput")
    idx_in = nc.dram_tensor("idx_in", [P, kw_tot], I16, kind="ExternalInput")
    dloc_in = nc.dram_tensor("dloc_in", [P, T * kc], F32, kind="ExternalInput")
    w1_in = nc.dram_tensor("w1_in", [P, KI * HID], F32, kind="ExternalInput")
    wmulv_in = nc.dram_tensor("wmulv_in", [HID, 2 * LAT], F32,
                             kind="ExternalInput")
    b1_in = nc.dram_tensor("b1_in", [HID, 1], F32, kind="ExternalInput")
    bmu_in = nc.dram_tensor("bmu_in", [LAT, 1], F32, kind="ExternalInput")
    blv_in = nc.dram_tensor("blv_in", [LAT, 1], F32, kind="ExternalInput")

    ODT = BF16 if OUT_BF16 else F32
    recon_out = nc.dram_tensor("recon_out", [L, N], ODT, kind="ExternalOutput")
    muT_out = nc.dram_tensor("muT_out", [LAT, L], F32, kind="ExternalOutput")
    lvT_out = nc.dram_tensor("lvT_out", [LAT, L], F32, kind="ExternalOutput")

    groups = [list(range(M))]

    with tile.TileContext(nc) as tc:
        with (
            tc.tile_pool(name="const", bufs=1) as cst,
            tc.tile_pool(name="dram", bufs=1, space="DRAM") as dram,
        ):
            # ---------- constants / persistents ----------
            identity = cst.tile([P, P], F32)
            make_identity(nc, identity[:])
            iota_i = cst.tile([P, P], I32)
            nc.gpsimd.iota(iota_i[:], pattern=[[1, P]], base=0,
                           channel_multiplier=0)
            iota_f = cst.tile([P, P], F32)
            nc.vector.tensor_copy(iota_f[:], iota_i[:])

            w1 = cst.tile([P, KI, HID], F32)   # w1[p,k,:] = W1[k*128+p,:]
            nc.sync.dma_start(
                w1[:], w1_in[:].rearrange("p (k h) -> p k h", k=KI))
            wmulv = cst.tile([HID, 2 * LAT], F32)
            nc.sync.dma_start(wmulv[:], wmulv_in[:])
            b1c = cst.tile([HID, 1], F32)
            nc.sync.dma_start(b1c[:], b1_in[:])
            bmuc = cst.tile([LAT, 1], F32)
            nc.sync.dma_start(bmuc[:], bmu_in[:])
            blvc = cst.tile([LAT, 1], F32)
            nc.sync.dma_start(blvc[:], blv_in[:])
            blvh = cst.tile([LAT, 1], F32)
            nc.vector.tensor_scalar_mul(blvh[:], blvc[:], 0.5)

            dinv = cst.tile([P, L], F32)
            idxs = cst.tile([P, kw_tot], I16)
            nc.sync.dma_start(idxs[:], idx_in[:])
            dloc = cst.tile([P, T * kc], F32)
            nc.sync.dma_start(dloc[:], dloc_in[:])

            xsT = cst.tile([P, L], F32)    # dinv * (x @ W1), feature-major
            hsT = cst.tile([P, L], F32)    # dinv * h, feature-major
            zT = cst.tile([LAT, L], F32)

            # DRAM: AllGather bounce buffers
            TDT = BF16 if TABLE_BF16 else F32
            xs_ag_in = dram.tile([L, HID], TDT)
            xs_full = dram.tile([N, HID], TDT, addr_space="Shared")
            hs_ag_in = dram.tile([L, HID], TDT)
            hs_full = dram.tile([N, HID], TDT, addr_space="Shared")
            ZDT = BF16 if RECON_BF16 else F32
            z_ag_in = dram.tile([LAT, L], ZDT)
            z_ag_out = dram.tile([M * LAT, L], ZDT, addr_space="Shared")

            # ---------- phase 1: Xs = dinv * (x @ W1); AllGather ----------
            with (
                tc.tile_pool(name="ph1", bufs=1) as p1,
                tc.tile_pool(name="wk1", bufs=3) as wk,
                tc.tile_pool(name="ps1", bufs=2, space="PSUM") as ps,
            ):
                deg = p1.tile([P, L], F32)
                nc.sync.dma_start(deg[:], deg_in[:])
                nc.scalar.activation(deg[:], deg[:],
                                     mybir.ActivationFunctionType.Sqrt)
                nc.vector.reciprocal(dinv[:], deg[:])
                xT = p1.tile([P, KI, L], F32)
                nc.sync.dma_start(
                    xT[:], xT_in[:].rearrange("p (k n) -> p k n", k=KI))
                for t in range(T):
                    cols = slice(t * P, (t + 1) * P)
                    xt_ps = ps.tile([P, P], F32, tag="mm")
                    for k in range(KI):
                        nc.tensor.matmul(
                            xt_ps[:], w1[:, k, :], xT[:, k, cols],
                            start=(k == 0), stop=(k == KI - 1))
                    nc.vector.tensor_tensor(out=xsT[:, cols], in0=xt_ps[:],
                                            in1=dinv[:, cols],
                                            op=mybir.AluOpType.mult)
                    # node-major copy for the gather table
                    nm_ps = ps.tile([P, P], F32, tag="tr")
                    nc.tensor.transpose(nm_ps[:], xsT[:, cols], identity[:])
                    nm = wk.tile([P, P], TDT, tag="nm")
                    nc.vector.tensor_copy(nm[:], nm_ps[:])
                    nc.sync.dma_start(xs_ag_in[t * P:(t + 1) * P, :], nm[:])
                nc.gpsimd.collective_compute(
                    "AllGather", mybir.AluOpType.bypass, replica_groups=groups,
                    ins=[xs_ag_in[:].opt()], outs=[xs_full[:].opt()])

            # ---------- phases 2+3: aggregations ----------
            with (
                tc.tile_pool(name="ph23", bufs=1) as p23,
                tc.tile_pool(name="gath", bufs=3) as gp,
                tc.tile_pool(name="wk23", bufs=3) as wk,
                tc.tile_pool(name="ps23", bufs=2, space="PSUM") as ps,
            ):
                epsT = p23.tile([LAT, L], F32)
                nc.sync.dma_start(epsT[:], epsT_in[:])
                muT = p23.tile([LAT, L], F32)
                lvT = p23.tile([LAT, L], F32)

                scache = p23.tile([P, T * kc, P], TDT)

                def sel_build(col):
                    """S[e, d] = (dloc[e] == d) for one 128-edge chunk."""
                    s = scache[:, col, :]
                    nc.vector.tensor_tensor(
                        out=s,
                        in0=dloc[:, col:col + 1].to_broadcast([P, P]),
                        in1=iota_f[:], op=mybir.AluOpType.is_equal)
                    return s

                # phase 2: h = relu(gcn1); Hs = dinv*h; AllGather
                for t in range(T):
                    cols = slice(t * P, (t + 1) * P)
                    g = gp.tile([P, kc, HID], TDT, tag="gather")
                    if t < 3:
                        nc.vector.memset(g[:], 0)  # first use of pool slot
                    for si, i0 in enumerate(range(0, ks[t], G_SUB)):
                        cnt = min(G_SUB, ks[t] - i0)
                        nc.gpsimd.dma_gather(
                            g[:, i0 // P:i0 // P + (-(-cnt // P)), :],
                            xs_full[:],
                            idxs[:, iofs[t] + i0 // 16:
                                 iofs[t] + (i0 + cnt) // 16],
                            cnt, cnt, HID, queue_num=(t * 8 + si) % N_QUEUES)
                    agg = ps.tile([P, P], F32, tag="mm")
                    for k in range(kcs[t]):
                        s = sel_build(t * kc + k)
                        nc.tensor.matmul(agg[:], g[:, k, :], s,
                                         start=(k == 0),
                                         stop=(k == kcs[t] - 1))
                    t1 = wk.tile([P, P], F32, tag="t1")
                    nc.vector.tensor_tensor(out=t1[:], in0=agg[:],
                                            in1=xsT[:, cols],
                                            op=mybir.AluOpType.add)
                    nc.vector.tensor_tensor(out=t1[:], in0=t1[:],
                                            in1=dinv[:, cols],
                                            op=mybir.AluOpType.mult)
                    ht = wk.tile([P, P], F32, tag="ht")
                    nc.scalar.activation(ht[:], t1[:],
                                         mybir.ActivationFunctionType.Relu,
                                         bias=b1c[:])
                    nc.vector.tensor_tensor(out=hsT[:, cols], in0=ht[:],
                                            in1=dinv[:, cols],
                                            op=mybir.AluOpType.mult)
                    nm_ps = ps.tile([P, P], F32, tag="tr")
                    nc.tensor.transpose(nm_ps[:], hsT[:, cols], identity[:])
                    nm = wk.tile([P, P], TDT, tag="nm")
                    nc.vector.tensor_copy(nm[:], nm_ps[:])
                    nc.sync.dma_start(hs_ag_in[t * P:(t + 1) * P, :], nm[:])
                nc.gpsimd.collective_compute(
                    "AllGather", mybir.AluOpType.bypass, replica_groups=groups,
                    ins=[hs_ag_in[:].opt()], outs=[hs_full[:].opt()])

                # phase 3: mu / logvar / z
                for t in range(T):
                    cols = slice(t * P, (t + 1) * P)
                    g = gp.tile([P, kc, HID], TDT, tag="gather")
                    if T + t < 3:
                        nc.vector.memset(g[:], 0)  # first use of pool slot
                    for si, i0 in enumerate(range(0, ks[t], G_SUB)):
                        cnt = min(G_SUB, ks[t] - i0)
                        nc.gpsimd.dma_gather(
                            g[:, i0 // P:i0 // P + (-(-cnt // P)), :],
                            hs_full[:],
                            idxs[:, iofs[t] + i0 // 16:
                                 iofs[t] + (i0 + cnt) // 16],
                            cnt, cnt, HID, queue_num=(t * 8 + si) % N_QUEUES)
                    agg = ps.tile([P, P], F32, tag="mm")
                    for k in range(kcs[t]):
                        s = scache[:, t * kc + k, :]
                        nc.tensor.matmul(agg[:], g[:, k, :], s,
                                         start=(k == 0),
                                         stop=(k == kcs[t] - 1))
                    gt = wk.tile([P, P], F32, tag="t1")
                    nc.vector.tensor_tensor(out=gt[:], in0=agg[:],
                                            in1=hsT[:, cols],
                                            op=mybir.AluOpType.add)
                    nc.vector.tensor_tensor(out=gt[:], in0=gt[:],
                                            in1=dinv[:, cols],
                                            op=mybir.AluOpType.mult)
                    mulv_ps = ps.tile([2 * LAT, P], F32, tag="mulv")
                    nc.tensor.matmul(mulv_ps[:], wmulv[:], gt[:], start=True,
                                     stop=True)
                    nc.vector.tensor_scalar_add(muT[:, cols],
                                                mulv_ps[:LAT, :], bmuc[:])
                    nc.vector.tensor_scalar_add(lvT[:, cols],
                                                mulv_ps[LAT:, :], blvc[:])
                    sd = wk.tile([LAT, P], F32, tag="sd")
                    nc.scalar.activation(sd[:], mulv_ps[LAT:, :],
                                         mybir.ActivationFunctionType.Exp,
                                         bias=blvh[:], scale=0.5)
                    nc.vector.tensor_tensor(out=sd[:], in0=epsT[:, cols],
                                            in1=sd[:],
                                            op=mybir.AluOpType.mult)
                    nc.vector.tensor_tensor(out=zT[:, cols],
                                            in0=muT[:, cols], in1=sd[:],
                                            op=mybir.AluOpType.add)
                nc.sync.dma_start(muT_out[:], muT[:])
                nc.sync.dma_start(lvT_out[:], lvT[:])
                if RECON_BF16:
                    zc = cst.tile([LAT, L], BF16)
                    nc.vector.tensor_copy(zc[:], zT[:])
                else:
                    zc = zT
                nc.sync.dma_start(z_ag_in[:], zc[:])
                nc.gpsimd.collective_compute(
                    "AllGather", mybir.AluOpType.bypass, replica_groups=groups,
                    ins=[z_ag_in[:].opt()], outs=[z_ag_out[:].opt()])

            # ---------- phase 4: recon = sigmoid(z @ z.T) block rows ------
            with (
                tc.tile_pool(name="ph4", bufs=1) as p4,
                tc.tile_pool(name="stage", bufs=2) as stg,
                tc.tile_pool(name="ps4", bufs=2, space="PSUM") as ps,
            ):
                zT_full = p4.tile([LAT, N], ZDT)
                nc.sync.dma_start(
                    zT_full[:].rearrange("f (r n) -> f r n", r=M),
                    z_ag_out[:].rearrange("(r f) n -> f r n", r=M))
                HALF = N // 2
                WID = min(2048, HALF)
                for t in range(T):
                    for half in range(2):
                        row = stg.tile([P, HALF], ODT, tag="stage")
                        for jj in range(HALF // WID):
                            r_ps = ps.tile([P, WID], F32, tag="recon")
                            for j in range(WID // 512):
                                c0 = half * HALF + jj * WID + j * 512
                                nc.tensor.matmul(
                                    r_ps[:, j * 512:(j + 1) * 512],
                                    zc[:, t * P:(t + 1) * P],
                                    zT_full[:, c0:c0 + 512],
                                    start=True, stop=True)
                            nc.scalar.activation(
                                row[:, jj * WID:(jj + 1) * WID], r_ps[:],
                                mybir.ActivationFunctionType.Sigmoid)
                        nc.sync.dma_start(
                            recon_out[t * P:(t + 1) * P,
                                      half * HALF:(half + 1) * HALF], row[:])
    nc.compile()
    return nc


def _shard(x, edge_index, eps, W1, b1, W_mu, b_mu, W_lv, b_lv):
    """Host-side sharding: per-core input maps + padded gather indices."""
    src = edge_index[0].astype(np.int64)
    dst = edge_index[1].astype(np.int64)
    deg = np.bincount(dst, minlength=N).astype(np.float32) + 1.0

    order = np.argsort(dst, kind="stable")
    src_s = src[order]
    dst_s = dst[order]
    gtile = dst_s // P                       # global dst tile of each edge
    counts = np.bincount(gtile, minlength=N // P)
    n_tiles = N // P
    # per tile SLOT t: max count over cores, padded to 16 (SPMD-shared)
    cpt = counts.reshape(M, T)
    ks = tuple(max(P, int(-(-(max(cpt[:, t].max(), K_PAD_MIN)) // 16)) * 16)
               for t in range(T))
    k_pad = max(int(-(-k // P)) * P for k in ks)

    src_pad = np.zeros((n_tiles, k_pad), np.int16)
    dloc_pad = np.full((n_tiles, k_pad), 999.0, np.float32)
    starts = np.zeros(n_tiles + 1, np.int64)
    np.cumsum(counts, out=starts[1:])
    for g in range(n_tiles):
        s, e = starts[g], starts[g + 1]
        src_pad[g, : e - s] = src_s[s:e]
        dloc_pad[g, : e - s] = (dst_s[s:e] % P).astype(np.float32)

    w1h = np.ascontiguousarray(
        W1.reshape(KI, P, HID).transpose(1, 0, 2).reshape(P, KI * HID))

    in_maps = []
    for c in range(M):
        rows = slice(c * L, (c + 1) * L)
        tl = slice(c * T, (c + 1) * T)
        # idx wrap: element i of each tile at [i%16, i//16], tiles along free
        w = np.concatenate(
            [src_pad[c * T + t, :ks[t]].reshape(ks[t] // 16, 16).T
             for t in range(T)], axis=1)
        w = np.tile(np.ascontiguousarray(w), (8, 1))
        dl = dloc_pad[tl].reshape(T, k_pad // P, P).transpose(2, 0, 1)
        dl = np.ascontiguousarray(dl.reshape(P, T * (k_pad // P)))
        xt = np.ascontiguousarray(
            x[rows].T.reshape(KI, P, L).transpose(1, 0, 2).reshape(P, KI * L))
        in_maps.append({
            "xT_in": xt,
            "epsT_in": np.ascontiguousarray(eps[rows].T),
            "deg_in": np.broadcast_to(deg[rows], (P, L)).copy(),
            "idx_in": w,
            "dloc_in": dl,
            "w1_in": w1h,
            "wmulv_in": np.ascontiguousarray(
                np.concatenate([W_mu, W_lv], axis=1)),
            "b1_in": np.ascontiguousarray(b1.reshape(HID, 1)),
            "bmu_in": np.ascontiguousarray(b_mu.reshape(LAT, 1)),
            "blv_in": np.ascontiguousarray(b_lv.reshape(LAT, 1)),
        })
    return in_maps, ks


def kernel(x, edge_index, eps, W1, b1, W_mu, b_mu, W_lv, b_lv, trace=False):
    in_maps, ks = _shard(
        np.asarray(x, np.float32), np.asarray(edge_index),
        np.asarray(eps, np.float32), np.asarray(W1, np.float32),
        np.asarray(b1, np.float32), np.asarray(W_mu, np.float32),
        np.asarray(b_mu, np.float32), np.asarray(W_lv, np.float32),
        np.asarray(b_lv, np.float32))
    if ks not in _cache:
        _cache[ks] = _build_program(ks)
    nc = _cache[ks]
    res = run_bass_kernel_spmd(nc, in_maps, core_ids=list(range(M)),
                               trace=trace)
    recon = np.concatenate([r["recon_out"] for r in res.results],
                           axis=0).astype(np.float32)
    mu = np.concatenate([r["muT_out"].T for r in res.results], axis=0)
    lv = np.concatenate([r["lvT_out"].T for r in res.results], axis=0)
    kernel.last_exec_time_ns = res.exec_time_ns
    return recon, mu, lv
